# revision 36
# baseline (speedup 1.0000x reference)
"""Multi-head causal attention (B=2, S=2048, D=1024, H=16, hd=64) on 8 TRN2
NeuronCores.

Sharding: 2-way batch x 4-way head tensor parallel. Core c handles batch
c//4 and heads 4*(c%4) .. 4*(c%4)+3 (a 256-column feature slice of the QKV
projections / 256 rows of o_W). Each core computes a full [2048, 1024]
partial of its batch's output; the host sums the 4 partials per batch.

Structure (all matmuls fp16, fp32 PSUM accumulate; ~146us HW, vs the
165.7us phase-separated version this evolved from):
  - Input DMA on the two HWDGE queues (sync + scalar), per-token-chunk
    SBUF tiles (precise dependency intervals), ordered so the first Q
    projection's operands (wq + x tokens 0:512) complete first. The
    startup is device-HBM-bound: all 8 cores burst-load concurrently.
  - PE warm-up matmuls bridge the input-DMA wait so the p-state ramp
    (half clock for the first ~3us of PE busy) is spent on throwaway
    work and the projections start at full clock.
  - Q/K/V projection for token chunk t+1 is pumped into the attention
    stream of query chunk t, so ACT (exp) gets work within ~15us of
    start and the PE never drains between phases.
  - Attention per (query chunk j, head pair): scores transposed
    St[k, q] = Kt.T @ Qt (the two heads' matmuls occupy complementary
    64-row PE tiles), exp via ACT (scale fused, one ACTIVATE per pair),
    causal mask via gpsimd affine_select, AV row-major with a
    ones-column appended to V so the same matmul produces the softmax
    denominator. AV is LDWEIGHTS-bound (~53ns per 65-col matmul).
  - Per-128-token output streaming: transpose + o-projection + fp16 y
    store emitted as each query subchunk's normalization finishes (the
    host sums 4 fp16 partials per batch in fp32).
Measured dead ends kept out of the design: fp8 data paths (e4m3 tails
bust the 2e-2 budget; fp8 LDWEIGHTS is column-rate-limited anyway),
64x64 QK quadrant tiling (2x LDWEIGHTS traffic loses ~8us), per-head
exp splits (ACT per-instruction overhead), gated/staged bulk DMA.
"""

import numpy as np

import concourse.mybir as mybir
import concourse.tile as tile
from concourse import bacc
from concourse.bass_utils import run_bass_kernel_spmd

F32 = mybir.dt.float32
F16 = mybir.dt.float16
F8 = mybir.dt.float8e4

S = 2048          # tokens per batch (= per core)
D = 1024          # model dim
HD = 64           # head dim
CORE_HEADS = 4    # heads per core
CF = CORE_HEADS * HD  # feature columns per core (256)
QC = 512          # query chunk (QK/exp granularity)
KC = 128          # key chunk
NQ = S // QC      # 4 query chunks
NK = S // KC      # 16 key chunks
ND = D // 128     # 8 contraction chunks

U8 = False        # fp8e4 exp output (no LDWEIGHTS win on HW; keep fp16)

_CACHE = {}


def build_nc(u8=U8, qk_tile=False):
    udt = F8 if u8 else F16
    ubias = float(-3.0 * np.log(2.0)) if u8 else 0.0
    nc = bacc.Bacc()
    xT = nc.dram_tensor("xT", [D, S], F16, kind="ExternalInput")
    wq = nc.dram_tensor("wq", [D, CF], F16, kind="ExternalInput")
    wk = nc.dram_tensor("wk", [D, CF], F16, kind="ExternalInput")
    wv = nc.dram_tensor("wv", [D, CF], F16, kind="ExternalInput")
    wo = nc.dram_tensor("wo", [CF, D], F16, kind="ExternalInput")
    y = nc.dram_tensor("y", [S, D], F16, kind="ExternalOutput")

    with tile.TileContext(nc) as tc:
        with (
            tc.tile_pool(name="big", bufs=1) as big,
            tc.tile_pool(name="w", bufs=1) as wpool,
            tc.tile_pool(name="u", bufs=34) as upool,
            tc.tile_pool(name="aoq", bufs=16) as aoqpool,
            tc.tile_pool(name="aot", bufs=3) as aotpool,
            tc.tile_pool(name="sm", bufs=8) as smpool,
            tc.tile_pool(name="ost", bufs=4) as ostpool,
            tc.tile_pool(name="ps", bufs=2, space="PSUM") as psp,
            tc.tile_pool(name="pav", bufs=2, space="PSUM") as pavp,
            tc.tile_pool(name="pt", bufs=2, space="PSUM") as ptp,
        ):
            # ---- constants ----
            if u8:
                ub_ap = wpool.tile([128, 1], F32, tag="ubias")
                nc.gpsimd.memset(ub_ap[:], ubias)
            ident = wpool.tile([128, 128], F16, tag="ident")
            nc.gpsimd.memset(ident[:], 0.0)
            nc.gpsimd.affine_select(
                out=ident[:], in_=ident[:],
                compare_op=mybir.AluOpType.not_equal, fill=1.0,
                base=0, channel_multiplier=1, pattern=[[-1, 128]],
            )

            # ---- weight + activation loads ----
            # Few large DMAs (issue costs ~600ns each), ordered so the Q
            # projection of token chunk 0 can start first: x tokens 0:512
            # stream on the gpsimd queue while wq streams on sync; the
            # remaining token chunks arrive one attention-unit ahead of the
            # projection that consumes them.
            wq_sb = wpool.tile([128, ND, CF], F16, tag="wq")
            wk_sb = wpool.tile([128, ND, CF], F16, tag="wk")
            wv_sb = wpool.tile([128, ND, CF], F16, tag="wv")
            wo_sb = wpool.tile([128, 2, D], F16, tag="wo")
            # One SBUF tile per token chunk: dependency tracking uses
            # per-partition bounding byte-intervals, so column-slice writes
            # into one big tile would serialize every consumer behind the
            # LAST x DMA. Separate tiles make the tracking precise.
            xs = [big.tile([128, ND, QC], F16, tag=f"xs{t}", name=f"xs{t}")
                  for t in range(NQ)]
            xTv = xT.rearrange("(n p) m -> p n m", p=128)

            # Critical loads first — the HBM queues aggregate ~460GB/s
            # across both issuing engines, so the bulk x chunks are held
            # back (see the gate writes below) until the first projection
            # is underway rather than stealing bandwidth from wq/xs0.
            wqv = wq.rearrange("(n p) m -> p n m", p=128)
            nc.scalar.dma_start(xs[0][:, 4:8, :], xTv[:, 4:8, 0:QC])
            nc.sync.dma_start(xs[0][:, 0:4, :], xTv[:, 0:4, 0:QC])
            nc.scalar.dma_start(wq_sb[:, 4:8, :], wqv[:, 4:8, :])
            nc.sync.dma_start(wq_sb[:, 0:4, :], wqv[:, 0:4, :])
            nc.scalar.dma_start(wk_sb[:], wk.rearrange("(n p) m -> p n m", p=128))
            nc.sync.dma_start(wv_sb[:], wv.rearrange("(n p) m -> p n m", p=128))
            nc.scalar.dma_start(xs[1][:], xTv[:, :, QC:2 * QC])
            nc.sync.dma_start(xs[2][:], xTv[:, :, 2 * QC:3 * QC])
            nc.scalar.dma_start(wo_sb[:], wo.rearrange("(b p) n -> p b n", p=128))
            nc.sync.dma_start(xs[3][:], xTv[:, :, 3 * QC:])

            # ---- PE warm-up ----
            # The PE runs at half clock for its first ~3us of busy time
            # (p-state ramp) and resets on idle gaps. Burn the input-DMA
            # wait on dependency-light matmuls (memset scratch, no gpsimd
            # dep) so the real projections start at full clock. Few long
            # matmuls, not many short ones — the PE sequencer's ~140ns
            # per-instruction decode would otherwise clog dispatch.
            warm_mov = wpool.tile([128, QC], F16, tag="warm_mov")
            warm_sb = wpool.tile([128, QC], F16, tag="warm")
            nc.vector.memset(warm_mov[:], 0.25)
            NWARM = 28
            ps_w = ptp.tile([128, QC], F32, tag="t", name="ps_warm")
            with nc.named_scope("mm_warm"):
                for n in range(NWARM):
                    nc.tensor.matmul(ps_w[:], warm_mov[:, 0:128], warm_mov[:],
                                     start=(n == 0), stop=(n == NWARM - 1))
            nc.vector.tensor_copy(warm_sb[:], ps_w[:])

            # ---- Q/K/V projections ----
            # qt/kt: [128, 2, S]: partition = feat % 128 (2 heads), block =
            # feat // 128 (head pair), col = token.
            qt = big.tile([128, 2, S], F16, tag="qt")
            kt = big.tile([128, 2, S], F16, tag="kt")
            v_sb = big.tile([128, NK, CORE_HEADS * (HD + 1)], F16, tag="v")
            nc.vector.memset(
                v_sb[:].rearrange("p n (h c) -> p n h c", c=HD + 1)[:, :, :, HD:],
                1.0,
            )

            def emit_proj(t):
                """Generator: QKV projection matmuls for token chunk t. At
                t=0 the contraction starts with chunks 4:8 (whose x/wq DMAs
                land first) so the PE can start ~3us earlier."""
                ks = [4, 5, 6, 7, 0, 1, 2, 3] if t == 0 else list(range(ND))
                for f in range(2):
                    ps_q = ptp.tile([128, QC], F32, tag="t", name=f"pq{t}_{f}")
                    ps_k = ptp.tile([128, QC], F32, tag="t", name=f"pk{t}_{f}")
                    with nc.named_scope("mm_projqk"):
                        for n, k in enumerate(ks):
                            nc.tensor.matmul(
                                ps_q[:],
                                wq_sb[:, k, 128 * f:128 * (f + 1)],
                                xs[t][:, k, :],
                                start=(n == 0), stop=(n == ND - 1),
                            )
                    yield
                    nc.vector.tensor_copy(qt[:, f, QC * t:QC * (t + 1)], ps_q[:])
                    with nc.named_scope("mm_projqk"):
                        for n, k in enumerate(ks):
                            nc.tensor.matmul(
                                ps_k[:],
                                wk_sb[:, k, 128 * f:128 * (f + 1)],
                                xs[t][:, k, :],
                                start=(n == 0), stop=(n == ND - 1),
                            )
                    yield
                    nc.vector.tensor_copy(kt[:, f, QC * t:QC * (t + 1)], ps_k[:])
                for tt in range(4 * t, 4 * t + 4):
                    ps = ptp.tile([128, CF], F32, tag="t")
                    with nc.named_scope("mm_projv"):
                        for n, k in enumerate(ks):
                            nc.tensor.matmul(
                                ps[:],
                                xs[t][:, k, KC * (tt % 4):KC * (tt % 4 + 1)],
                                wv_sb[:, k, :],
                                start=(n == 0), stop=(n == ND - 1),
                            )
                    yield
                    nc.vector.tensor_copy(
                        v_sb[:, tt, :].rearrange("p (h c) -> p h c", c=HD + 1)[:, :, :HD],
                        ps[:].rearrange("p (h c) -> p h c", c=HD),
                    )

            # ---- attention + output projection ----
            # Software-pipelined across (query-chunk, head-pair) units: the
            # AV matmuls of unit k-1 (pure PE work) are interleaved with the
            # QK+exp phase of unit k (ACT-paced) so the PE never waits on
            # the scalar engine. The projection of token chunk j+1 is pumped
            # into the same stream.
            units = [(j, pair) for j in range(NQ) for pair in range(2)]
            us = {}
            ao_q = {}

            def emit_A(unit, i):
                # Both heads' scores land in one 2-bank PSUM tile so a
                # single ACTIVATE (and a single affine_select) covers the
                # pair — halves the fixed ACT pipeline overhead.
                j, pair = unit
                t = i - 4 * j
                qo = max(0, KC * t)
                w = QC - qo
                ps_s = psp.tile([128, 2, QC], F32, tag="s", name=f"s{j}_{pair}_{i}")
                for hx, h in enumerate((2 * pair, 2 * pair + 1)):
                    hp = 64 * (h % 2)
                    if qk_tile:
                        for kh in range(2):
                            with nc.named_scope("mm_qk"):
                                nc.tensor.matmul(
                                    ps_s[64 * kh:64 * (kh + 1), hx, 0:w],
                                    kt[hp:hp + 64, pair,
                                       KC * i + 64 * kh:KC * i + 64 * (kh + 1)],
                                    qt[hp:hp + 64, pair,
                                       QC * j + qo:QC * (j + 1)],
                                    start=True, stop=True,
                                    skip_group_check=True,
                                    tile_position=(hp, 64 * kh),
                                )
                    else:
                        with nc.named_scope("mm_qk"):
                            nc.tensor.matmul(
                                ps_s[:, hx, 0:w],
                                kt[hp:hp + 64, pair, KC * i:KC * (i + 1)],
                                qt[hp:hp + 64, pair, QC * j + qo:QC * (j + 1)],
                                start=True, stop=True,
                                skip_group_check=True,
                            )
                u = upool.tile([128, 2, w], udt, tag="u", name=f"u{j}_{pair}_{i}")
                nc.scalar.activation(
                    u[:], ps_s[:, :, 0:w],
                    mybir.ActivationFunctionType.Exp, scale=0.125,
                    bias=ub_ap[:] if u8 else 0.0,
                )
                if t >= 0:
                    nc.gpsimd.affine_select(
                        out=u[:, :, 0:KC], in_=u[:, :, 0:KC],
                        compare_op=mybir.AluOpType.is_ge, fill=0.0,
                        base=0, channel_multiplier=-1,
                        pattern=[[0, 2], [1, KC]],
                    )
                us[unit, i] = (u, qo)

            def emit_out_qq(j, qq):
                """Transpose + output projection + store for one 128-token
                query subchunk (all 4 heads of this core)."""
                aot = aotpool.tile([128, 2, KC], F16, tag="aot",
                                   name=f"aot{j}_{qq}")
                for b in range(2):
                    ps_t = ptp.tile([128, 128], F16, tag="t",
                                    name=f"pt{j}_{qq}_{b}")
                    with nc.named_scope("mm_tpose"):
                        nc.tensor.transpose(
                            ps_t[:], ao_q[j][qq][:, 128 * b:128 * (b + 1)],
                            ident[:],
                        )
                    nc.vector.tensor_copy(aot[:, b, :], ps_t[:])
                ps_o = [ptp.tile([128, QC], F32, tag="t",
                                 name=f"po{j}_{qq}_{n}")
                        for n in range(2)]
                with nc.named_scope("mm_oproj"):
                    for b in range(2):
                        for n in range(2):
                            nc.tensor.matmul(
                                ps_o[n][:],
                                aot[:, b, :],
                                wo_sb[:, b, 512 * n:512 * (n + 1)],
                                start=(b == 0), stop=(b == 1),
                            )
                ost = ostpool.tile([128, D], F16, tag="ost",
                                   name=f"ost{j}_{qq}")
                for n in range(2):
                    nc.vector.tensor_copy(ost[:, 512 * n:512 * (n + 1)],
                                          ps_o[n][:])
                # mid-stream chunks ride the slow gpsimd queue; the final
                # query chunk uses the fast hwdge queues to keep the tail
                # short (scalar's exp work is finished by then).
                if j < NQ - 1:
                    eng = nc.sync if qq % 2 == 0 else nc.gpsimd
                else:
                    eng = nc.sync if qq % 2 == 0 else nc.scalar
                eng.dma_start(
                    y[QC * j + KC * qq:QC * j + KC * (qq + 1), :], ost[:],
                )

            def emit_B(unit):
                """Generator: AV matmuls for one unit, yielding after each
                key-chunk step; norms at each query-subchunk's end. For the
                second head pair, the finished subchunk's output projection
                is emitted immediately so y streams out incrementally.
                One PSUM bank per accumulation group (bank-granular
                start/stop semantics)."""
                j, pair = unit
                nk = 4 * j + 4
                if j not in ao_q:
                    ao_q[j] = [aoqpool.tile([128, CF], F16, tag="aoq",
                                            name=f"ao_q{j}_{qq}")
                               for qq in range(4)]
                for qq in range(4):
                    for hx, h in enumerate((2 * pair, 2 * pair + 1)):
                        av = pavp.tile([128, HD + 1], F32, tag="av",
                                       name=f"av{j}_{h}_{qq}")
                        last = 4 * j + qq
                        with nc.named_scope("mm_av"):
                            for i in range(last + 1):
                                u, qo = us[unit, i]
                                nc.tensor.matmul(
                                    av[:],
                                    u[:, hx, KC * qq - qo:KC * (qq + 1) - qo],
                                    v_sb[:, i, 65 * h:65 * h + 65],
                                    start=(i == 0), stop=(i == last),
                                )
                                yield
                        rec = smpool.tile([128, 1], F32, tag="rec",
                                          name=f"rec{j}_{h}_{qq}")
                        nc.vector.reciprocal(rec[:], av[:, HD:HD + 1])
                        nc.vector.tensor_scalar_mul(
                            ao_q[j][qq][:, HD * h:HD * (h + 1)],
                            av[:, 0:HD], rec[:],
                        )
                    if pair == 1:
                        emit_out_qq(j, qq)
                    yield
                for i in range(nk):
                    us.pop((unit, i), None)

            from collections import deque

            proj_q = deque()

            def pump():
                while proj_q:
                    if next(proj_q[0], "done") == "done":
                        proj_q.popleft()
                        continue
                    return

            # Q/K of token chunk 0 run standalone (the first attention unit
            # needs them); V of chunk 0 and all of chunk j+1 are pumped into
            # query chunk j's attention stream so exp starts ASAP.
            g0 = emit_proj(0)
            for _ in range(4):
                next(g0)
            proj_q.append(g0)

            prev_gen = None
            for unit in units:
                j, pair = unit
                if pair == 0 and j + 1 < NQ:
                    proj_q.append(emit_proj(j + 1))
                nk_c = 4 * j + 4
                for ii in range(nk_c):
                    emit_A(unit, ii)
                    pump()
                    if prev_gen is not None:
                        for _ in range(2):
                            next(prev_gen, None)
                if prev_gen is not None:
                    for _ in prev_gen:
                        pump()
                prev_gen = emit_B(unit)
            for _ in prev_gen:
                pump()
            while proj_q:
                pump()
    nc.compile()
    return nc


def _get_nc():
    if "nc" not in _CACHE:
        _CACHE["nc"] = build_nc()
    return _CACHE["nc"]


def make_in_maps(x, q_W, k_W, v_W, o_W):
    x = np.asarray(x, dtype=np.float32)
    in_maps = []
    xTs = [np.ascontiguousarray(x[b].T).astype(np.float16) for b in range(2)]
    for c in range(8):
        b, g = c // 4, c % 4
        fs = slice(CF * g, CF * (g + 1))
        in_maps.append({
            "xT": xTs[b],
            "wq": np.ascontiguousarray(np.asarray(q_W, np.float32)[fs].T).astype(np.float16),
            "wk": np.ascontiguousarray(np.asarray(k_W, np.float32)[fs].T).astype(np.float16),
            "wv": np.ascontiguousarray(np.asarray(v_W, np.float32)[fs].T).astype(np.float16),
            "wo": np.ascontiguousarray(np.asarray(o_W, np.float32)[:, fs].T).astype(np.float16),
        })
    return in_maps


def kernel(x, q_W, k_W, v_W, o_W, trace=False):
    nc = _get_nc()
    in_maps = make_in_maps(x, q_W, k_W, v_W, o_W)
    res = run_bass_kernel_spmd(nc, in_maps, core_ids=list(range(8)),
                               trace=trace)
    _CACHE["last_results"] = res
    ys = [res.results[c]["y"].astype(np.float32) for c in range(8)]
    out = np.stack([
        ys[0] + ys[1] + ys[2] + ys[3],
        ys[4] + ys[5] + ys[6] + ys[7],
    ]).astype(np.float32)
    return out
